# revision 11
# baseline (speedup 1.0000x reference)
"""Trainium2 Bass kernel: strided 3x3 conv (stride 2, pad 1) + bias
+ hardswish + mish, data-parallel over batch across 8 NeuronCores.

Shapes (hardcoded):
  x (16,64,256,256) f32; weight (128,64,3,3); bias (128,)
  out (16,128,128,128) f32

Design:
- Host pre-pads, de-interleaves and fp16-casts x into [128,257,257] per
  core (partition = 2 images x 64 cin): row 0 = top zero pad; per row:
  [128 even cols | 129 odd cols (leading left-pad zero)]. Every conv tap
  reads a CONTIGUOUS 128-wide slice; x DMAs move 2-chunk blocks (33 rows,
  ~17KB contiguous per partition) in ONE dma_start for both images.
- Conv = 9 fp16 tap-matmuls (fp32 PSUM accumulate) into ONE [128,2048]
  PSUM tile per 8-row chunk (img-major, 4 banks). The conv bias, the
  -0.5 shift and hardswish's +3 all ride the ACT bias vector (no bias
  tap matmul). Two images packed in PE row groups (partitions 0-63 /
  64-127, tile_position (0,0)/(64,0)); per tap the order g0:(A,B),
  g1:(A,B) keeps both halves streaming concurrently.
- Tail is 2 ACT + 4 DVE single-instruction passes per chunk, all fp16
  SBUF in accelerated DVE modes (no 1x scalar_tensor_tensor), with ONE
  PSUM read so banks free early:
    q   = Relu(y + b + 2.5)      [ACT, the only PSUM read]
    t6  = min(q,6)/6             [DVE ts 4x]
    qm3 = q - 3                  [DVE ts 4x]
    h6  = qm3*t6                 [DVE tt 2x]  == hardswish, exact
    T   = Sigmoid(g*h6 + a)      [ACT]  ~= tanh(softplus(h6)), fitted
    o   = T*h6                   [DVE tt 2x]  == mish(h6)
  (g,a) least-squares fitted; end-to-end fp16 norm-rel ~7e-3 (budget
  2e-2). sigmoid+relu live in one act table set -> single table load.
  T(c-1)/o(c-1) are emitted after the chunk-c DVE chain so neither ACT
  nor DVE ever stalls on the other.
- Output stays fp16: o written into a 4-chunk staging tile laid out
  [cout, (img, row32, col)] -> ONE HWDGE DMA per 4 chunks (8KB DRAM
  descriptors); host upcasts to fp32. Out DRAM is channel-major
  [COUT, PER, HO, WO]; host transposes on gather.
"""
import numpy as np

import concourse.bass as bass
import concourse.mybir as mybir
import concourse.tile as tile
from concourse import bacc
from concourse.bass_utils import run_bass_kernel_spmd

F32 = mybir.dt.float32
F16 = mybir.dt.float16
AFT = mybir.ActivationFunctionType
ALU = mybir.AluOpType

B, CIN, H, W = 16, 64, 256, 256
COUT = 128
HO, WO = 128, 128
NCORE = 8
PER = B // NCORE          # images per core
R = 8                     # output rows per chunk
NCHUNK = HO // R          # 16
WP = W + 1                # de-interleaved row width (128 even + 129 odd)
XROWS = 4 * R + 1         # input rows per 2-chunk x tile (33)

# mish(h) ~= h*sigmoid(SG*h + SA), fitted over h=hardswish(N(-0.5,1))
SG, SA = 1.41781445, 0.43675223

_CACHE: dict = {}

# inner-column offset into the de-interleaved row, per kj
_KJ_OFF = {0: 128, 1: 0, 2: 129}


def _build():
    nc = bacc.Bacc(None, target_bir_lowering=False)
    x_ext = nc.declare_dram_parameter("x", [PER * CIN, H + 1, WP], F16,
                                      isOutput=False)
    wt_ext = nc.declare_dram_parameter("wt", [128, 9 * COUT], F16,
                                       isOutput=False)
    ones_ext = nc.declare_dram_parameter("ones", [128, 512], F16,
                                         isOutput=False)
    bvec_ext = nc.declare_dram_parameter("bvec", [128, 1], F32,
                                         isOutput=False)
    out_ext = nc.declare_dram_parameter("out", [COUT, PER, HO, WO], F16,
                                        isOutput=True)

    N1 = R * WO            # 1024: one image-chunk
    N2 = PER * N1          # 2048: both images of a chunk

    with tile.TileContext(nc) as tc:
        with (
            tc.tile_pool(name="const", bufs=1) as cpool,
            tc.tile_pool(name="xin", bufs=3) as xpool,
            tc.tile_pool(name="work", bufs=2) as wpool,
            tc.tile_pool(name="h6p", bufs=3) as hpool,
            tc.tile_pool(name="tep", bufs=2) as tpool,
            tc.tile_pool(name="psum", bufs=2, space="PSUM") as ppool,
        ):
            wt_sb = cpool.tile([128, 9 * COUT], F16)
            nc.sync.dma_start(out=wt_sb[:], in_=wt_ext[:])
            ones_sb = cpool.tile([128, 512], F16)
            nc.sync.dma_start(out=ones_sb[:], in_=ones_ext[:])
            bvec_sb = cpool.tile([128, 1], F32)
            nc.sync.dma_start(out=bvec_sb[:], in_=bvec_ext[:])
            sa_sb = cpool.tile([128, 1], F32)
            nc.vector.memset(sa_sb[:], SA)

            # HAM warmup: ~7us of dummy matmuls so the PE clock is at
            # 2.4GHz for the real work and stays busy until the first
            # x DMA lands (no >3.4us idle gap -> no re-throttle).
            warm = ppool.tile([128, N2], F32, tag="pt", name="warm")
            for m in range(32):
                p0 = 64 * (m % 2)
                nc.tensor.matmul(
                    warm[:, (m % 2) * 512 : (m % 2) * 512 + 512],
                    wt_sb[p0 : p0 + 64, 0:COUT],
                    ones_sb[p0 : p0 + 64, :],
                    start=True, stop=True, tile_position=(p0, 0),
                )
            # consume the scratch (also triggers the one act-table load
            # for {sigmoid, relu} before the hot loop)
            wsink = cpool.tile([128, 8], F32)
            nc.scalar.activation(wsink[:], warm[:, 0:8], AFT.Sigmoid)

            te_box = [None]
            pending = []       # [(chunk, h6)] awaiting sigmoid+mult

            def _tail(m, h6):
                # T(m) = sigmoid(SG*h6+SA); o(m) = T*h6 -> te; dma per 4
                cc = m % 4
                T = wpool.tile([128, N2], F16, name=f"T{m}", tag="T")
                nc.scalar.activation(T[:], h6[:], AFT.Sigmoid,
                                     scale=SG, bias=sa_sb[:, 0:1])
                if cc == 0:
                    te_box[0] = tpool.tile([128, 4 * N2], F16, name="te")
                te = te_box[0]
                tev = te[:].rearrange("p (i c n) -> p i c n", i=PER, c=4)
                nc.vector.tensor_tensor(
                    tev[:, :, cc, :],
                    T[:].rearrange("p (i n) -> p i n", i=PER),
                    h6[:].rearrange("p (i n) -> p i n", i=PER),
                    ALU.mult,
                )
                if cc == 3:
                    # SWDGE: keeps the dma trigger's sem wait OFF the
                    # ACT queue (a scalar-engine trigger blocks q/T
                    # behind its wait); the gpsimd queue is idle.
                    g4 = m // 4
                    nc.gpsimd.dma_start(
                        out=out_ext[:, :, 32 * g4 : 32 * g4 + 32, :],
                        in_=te[:].rearrange("p (i r c) -> p i r c",
                                            i=PER, c=WO),
                    )

            for c in range(NCHUNK):
                if c % 2 == 0:
                    t2 = c // 2
                    xt = xpool.tile([128, XROWS * WP], F16)
                    xt3 = xt[:].rearrange("p (r c) -> p r c", c=WP)
                    nc.sync.dma_start(
                        out=xt3[:, :, :],
                        in_=x_ext[:, 32 * t2 : 32 * t2 + XROWS, :],
                    )
                rbase = 16 * (c % 2)

                pts = ppool.tile([128, N2], F32, tag="pt", name=f"pt{c}")
                for t in range(9):
                    for g in range(2):
                        for i in range(PER):
                            p0 = 64 * i
                            ki, kj = divmod(t, 3)
                            s = rbase + 8 * g + ki
                            off = _KJ_OFF[kj]
                            nc.tensor.matmul(
                                pts[:, i * N1 + g * 512 :
                                       i * N1 + (g + 1) * 512],
                                wt_sb[p0 : p0 + 64,
                                      t * COUT : (t + 1) * COUT],
                                xt3[p0 : p0 + 64, s : s + 7 : 2,
                                    off : off + WO],
                                start=(t == 0), stop=(t == 8),
                                tile_position=(p0, 0),
                            )

                # q = relu(y + b + 2.5); the ONLY psum read -> banks free
                q = wpool.tile([128, N2], F16, name="q", tag="q")
                nc.scalar.activation(q[:], pts[:], AFT.Relu,
                                     bias=bvec_sb[:, 0:1])
                # hardswish from q alone: h6 = (q-3)*min(q,6)/6
                t6 = wpool.tile([128, N2], F16, name="t6", tag="t6")
                nc.vector.tensor_scalar(t6[:], q[:], 6.0, 1.0 / 6.0,
                                        ALU.min, ALU.mult)
                qm3 = wpool.tile([128, N2], F16, name="qm3", tag="qm3")
                nc.vector.tensor_scalar(qm3[:], q[:], -3.0, None, ALU.add)
                h6 = hpool.tile([128, N2], F16, name="h6", tag="h6")
                nc.vector.tensor_tensor(h6[:], qm3[:], t6[:], ALU.mult)
                # sigmoid+mult tail, pipelined TWO chunks back: by the
                # time T(c-2) is scheduled its h6 is long since done, so
                # the ACT queue never blocks on the DVE chain (which
                # would otherwise put q -> h6 -> T -> next-q on the
                # critical cycle).
                pending.append((c, h6))
                if len(pending) > 2:
                    _tail(*pending.pop(0))
            for it in pending:
                _tail(*it)
    nc.compile()
    return nc


def _get_nc():
    if "nc" not in _CACHE:
        _CACHE["nc"] = _build()
    return _CACHE["nc"]


def _prep(x, weight, bias):
    x = np.asarray(x, dtype=np.float32)
    w = np.asarray(weight, dtype=np.float32)
    b = np.asarray(bias, dtype=np.float32)

    # de-interleave + pad + fp16: row 0 = top pad; cols [0:128]=even orig
    # cols, [128]=left pad, [129:257]=odd orig cols 1,3,...,255
    x_de = np.zeros((B, CIN, H + 1, WP), dtype=np.float16)
    x_de[:, :, 1:, 0:128] = x[:, :, :, 0::2]
    x_de[:, :, 1:, 129:257] = x[:, :, :, 1::2]
    x_de = x_de.reshape(NCORE, PER * CIN, H + 1, WP)

    # wt: [cin, tap*COUT], duplicated across both partition halves
    wt = np.ascontiguousarray(
        w.transpose(1, 2, 3, 0).reshape(CIN, 9 * COUT).astype(np.float16))
    wt2 = np.ascontiguousarray(np.concatenate([wt, wt], axis=0))

    ones = np.ones((128, 512), dtype=np.float16)
    # ACT bias: conv bias - 0.5 (SUBTRACT_VALUE) + 3 (hardswish shift)
    bvec = (b.astype(np.float64) + 2.5).astype(np.float32).reshape(128, 1)
    in_maps = [
        {"x": x_de[i], "wt": wt2, "ones": ones, "bvec": bvec}
        for i in range(NCORE)
    ]
    return in_maps


def _run(in_maps, **kw):
    nc = _get_nc()
    return run_bass_kernel_spmd(nc, in_maps, list(range(NCORE)), **kw)


def kernel(x, weight, bias):
    res = _run(_prep(x, weight, bias))
    out = np.empty((B, COUT, HO, WO), dtype=np.float32)
    for i in range(NCORE):
        # device result is [COUT, PER, HO, WO] fp16
        out[PER * i : PER * (i + 1)] = res.results[i]["out"].transpose(
            1, 0, 2, 3)
    return out


# revision 13
# speedup vs baseline: 1.0619x; 1.0619x over previous
"""Trainium2 Bass kernel: strided 3x3 conv (stride 2, pad 1) + bias
+ hardswish + mish, data-parallel over batch across 8 NeuronCores.

Shapes (hardcoded):
  x (16,64,256,256) f32; weight (128,64,3,3); bias (128,)
  out (16,128,128,128) f32

Design:
- Host pre-pads, de-interleaves and fp16-casts x into [128,257,257] per
  core (partition = 2 images x 64 cin): row 0 = top zero pad; per row:
  [128 even cols | 129 odd cols (leading left-pad zero)]. Every conv tap
  reads a CONTIGUOUS 128-wide slice; x DMAs move 2-chunk blocks (33 rows,
  ~17KB contiguous per partition) in ONE dma_start for both images.
- Conv = 9 fp16 tap-matmuls (fp32 PSUM accumulate) into ONE [128,2048]
  PSUM tile per 8-row chunk (img-major, 4 banks). The conv bias, the
  -0.5 shift and hardswish's +3 all ride the ACT bias vector (no bias
  tap matmul). Two images packed in PE row groups (partitions 0-63 /
  64-127, tile_position (0,0)/(64,0)); per tap the order g0:(A,B),
  g1:(A,B) keeps both halves streaming concurrently.
- Tail is 2 ACT + 4 DVE single-instruction passes per chunk, all fp16
  SBUF in accelerated DVE modes (no 1x scalar_tensor_tensor), with ONE
  PSUM read so banks free early:
    q   = Relu(y + b + 2.5)      [ACT, the only PSUM read]
    t6  = min(q,6)/6             [DVE ts 4x]
    qm3 = q - 3                  [DVE ts 4x]
    h6  = qm3*t6                 [DVE tt 2x]  == hardswish, exact
    T   = Sigmoid(g*h6 + a)      [ACT]  ~= tanh(softplus(h6)), fitted
    o   = T*h6                   [DVE tt 2x]  == mish(h6)
  (g,a) least-squares fitted; end-to-end fp16 norm-rel ~7e-3 (budget
  2e-2). sigmoid+relu live in one act table set -> single table load.
  T(c-1)/o(c-1) are emitted after the chunk-c DVE chain so neither ACT
  nor DVE ever stalls on the other.
- Output stays fp16: o written into a 4-chunk staging tile laid out
  [cout, (img, row32, col)] -> ONE HWDGE DMA per 4 chunks (8KB DRAM
  descriptors); host upcasts to fp32. Out DRAM is channel-major
  [COUT, PER, HO, WO]; host transposes on gather.
"""
import numpy as np

import concourse.bass as bass
import concourse.mybir as mybir
import concourse.tile as tile
from concourse import bacc
from concourse.bass_utils import run_bass_kernel_spmd

F32 = mybir.dt.float32
F16 = mybir.dt.float16
AFT = mybir.ActivationFunctionType
ALU = mybir.AluOpType

B, CIN, H, W = 16, 64, 256, 256
COUT = 128
HO, WO = 128, 128
NCORE = 8
PER = B // NCORE          # images per core
R = 8                     # output rows per chunk
NCHUNK = HO // R          # 16
WP = W + 1                # de-interleaved row width (128 even + 129 odd)
XROWS = 4 * R + 1         # input rows per 2-chunk x tile (33)

# mish(h) ~= h*sigmoid(SG*h + SA), fitted over h=hardswish(N(-0.5,1))
SG, SA = 1.41781445, 0.43675223

_CACHE: dict = {}

# inner-column offset into the de-interleaved row, per kj
_KJ_OFF = {0: 128, 1: 0, 2: 129}


def _build():
    nc = bacc.Bacc(None, target_bir_lowering=False)
    x_ext = nc.declare_dram_parameter("x", [PER * CIN, H + 1, WP], F16,
                                      isOutput=False)
    wt_ext = nc.declare_dram_parameter("wt", [128, 9 * COUT], F16,
                                       isOutput=False)
    ones_ext = nc.declare_dram_parameter("ones", [128, 512], F16,
                                         isOutput=False)
    bvec_ext = nc.declare_dram_parameter("bvec", [128, 1], F32,
                                         isOutput=False)
    out_ext = nc.declare_dram_parameter("out", [COUT, PER, HO, WO], F16,
                                        isOutput=True)

    N1 = R * WO            # 1024: one image-chunk
    N2 = PER * N1          # 2048: both images of a chunk

    with tile.TileContext(nc) as tc:
        with (
            tc.tile_pool(name="const", bufs=1) as cpool,
            tc.tile_pool(name="xin", bufs=3) as xpool,
            tc.tile_pool(name="work", bufs=2) as wpool,
            tc.tile_pool(name="h6p", bufs=3) as hpool,
            tc.tile_pool(name="tep", bufs=2) as tpool,
            tc.tile_pool(name="psum", bufs=2, space="PSUM") as ppool,
        ):
            wt_sb = cpool.tile([128, 9 * COUT], F16)
            nc.sync.dma_start(out=wt_sb[:], in_=wt_ext[:])
            ones_sb = cpool.tile([128, 512], F16)
            nc.sync.dma_start(out=ones_sb[:], in_=ones_ext[:])
            bvec_sb = cpool.tile([128, 1], F32)
            nc.sync.dma_start(out=bvec_sb[:], in_=bvec_ext[:])
            sa_sb = cpool.tile([128, 1], F32)
            nc.vector.memset(sa_sb[:], SA)

            # HAM warmup: ~7us of dummy matmuls so the PE clock is at
            # 2.4GHz for the real work and stays busy until the first
            # x DMA lands (no >3.4us idle gap -> no re-throttle).
            warm = ppool.tile([128, N2], F32, tag="pt", name="warm")
            for m in range(32):
                p0 = 64 * (m % 2)
                nc.tensor.matmul(
                    warm[:, (m % 2) * 512 : (m % 2) * 512 + 512],
                    wt_sb[p0 : p0 + 64, 0:COUT],
                    ones_sb[p0 : p0 + 64, :],
                    start=True, stop=True, tile_position=(p0, 0),
                )
            # consume the scratch (also triggers the one act-table load
            # for {sigmoid, relu} before the hot loop)
            wsink = cpool.tile([128, 8], F32)
            nc.scalar.activation(wsink[:], warm[:, 0:8], AFT.Sigmoid)

            te_box = [None]
            h2_box = [None]
            pending = []       # [pair] awaiting sigmoid+mult

            def _tail(g2, h2):
                # One sigmoid + one mult over a 2-chunk pair (N=4096):
                # fewer ACT instructions between the q's means the
                # psum-release chain (PE waits "ACT count >= pos(q)")
                # almost never waits on a T. Lagged one pair so h2 is
                # long since complete when T2 is scheduled.
                T2 = wpool.tile([128, 2 * N2], F16, name=f"T{g2}", tag="T")
                nc.scalar.activation(T2[:], h2[:], AFT.Sigmoid,
                                     scale=SG, bias=sa_sb[:, 0:1])
                if g2 % 2 == 0:
                    te_box[0] = tpool.tile([128, 4 * N2], F16, name="te")
                te = te_box[0]
                tev = te[:].rearrange("p (i c n) -> p i c n", i=PER, c=2)
                nc.vector.tensor_tensor(
                    tev[:, :, g2 % 2, :],
                    T2[:].rearrange("p (i n) -> p i n", i=PER),
                    h2[:].rearrange("p (i n) -> p i n", i=PER),
                    ALU.mult,
                )
                if g2 % 2 == 1:
                    # SWDGE: keeps the dma trigger's sem wait OFF the
                    # ACT queue (a scalar-engine trigger blocks q/T
                    # behind its wait); the gpsimd queue is idle.
                    g4 = g2 // 2
                    nc.gpsimd.dma_start(
                        out=out_ext[:, :, 32 * g4 : 32 * g4 + 32, :],
                        in_=te[:].rearrange("p (i r c) -> p i r c",
                                            i=PER, c=WO),
                    )

            for c in range(NCHUNK):
                if c % 2 == 0:
                    t2 = c // 2
                    xt = xpool.tile([128, XROWS * WP], F16)
                    xt3 = xt[:].rearrange("p (r c) -> p r c", c=WP)
                    nc.sync.dma_start(
                        out=xt3[:, :, :],
                        in_=x_ext[:, 32 * t2 : 32 * t2 + XROWS, :],
                    )
                rbase = 16 * (c % 2)

                pts = ppool.tile([128, N2], F32, tag="pt", name=f"pt{c}")
                for t in range(9):
                    for g in range(2):
                        for i in range(PER):
                            p0 = 64 * i
                            ki, kj = divmod(t, 3)
                            s = rbase + 8 * g + ki
                            off = _KJ_OFF[kj]
                            nc.tensor.matmul(
                                pts[:, i * N1 + g * 512 :
                                       i * N1 + (g + 1) * 512],
                                wt_sb[p0 : p0 + 64,
                                      t * COUT : (t + 1) * COUT],
                                xt3[p0 : p0 + 64, s : s + 7 : 2,
                                    off : off + WO],
                                start=(t == 0), stop=(t == 8),
                                tile_position=(p0, 0),
                            )

                # q = relu(y + b + 2.5); the ONLY psum read -> banks free
                q = wpool.tile([128, N2], F16, name="q", tag="q")
                nc.scalar.activation(q[:], pts[:], AFT.Relu,
                                     bias=bvec_sb[:, 0:1])
                # hardswish from q alone: h6 = (q-3)*min(q,6)/6
                t6 = wpool.tile([128, N2], F16, name="t6", tag="t6")
                nc.vector.tensor_scalar(t6[:], q[:], 6.0, 1.0 / 6.0,
                                        ALU.min, ALU.mult)
                qm3 = wpool.tile([128, N2], F16, name="qm3", tag="qm3")
                nc.vector.tensor_scalar(qm3[:], q[:], -3.0, None, ALU.add)
                # hardswish lands in the 2-chunk pair staging tile
                if c % 2 == 0:
                    h2_box[0] = hpool.tile([128, 2 * N2], F16, name="h2")
                h2 = h2_box[0]
                h2v = h2[:].rearrange("p (i s n) -> p i s n", i=PER, s=2)
                nc.vector.tensor_tensor(
                    h2v[:, :, c % 2, :],
                    qm3[:].rearrange("p (i n) -> p i n", i=PER),
                    t6[:].rearrange("p (i n) -> p i n", i=PER),
                    ALU.mult,
                )
                if c % 2 == 1:
                    pending.append((c // 2, h2))
                    if len(pending) > 1:
                        _tail(*pending.pop(0))
            for it in pending:
                _tail(*it)
    nc.compile()
    return nc


def _get_nc():
    if "nc" not in _CACHE:
        _CACHE["nc"] = _build()
    return _CACHE["nc"]


def _prep(x, weight, bias):
    x = np.asarray(x, dtype=np.float32)
    w = np.asarray(weight, dtype=np.float32)
    b = np.asarray(bias, dtype=np.float32)

    # de-interleave + pad + fp16: row 0 = top pad; cols [0:128]=even orig
    # cols, [128]=left pad, [129:257]=odd orig cols 1,3,...,255
    x_de = np.zeros((B, CIN, H + 1, WP), dtype=np.float16)
    x_de[:, :, 1:, 0:128] = x[:, :, :, 0::2]
    x_de[:, :, 1:, 129:257] = x[:, :, :, 1::2]
    x_de = x_de.reshape(NCORE, PER * CIN, H + 1, WP)

    # wt: [cin, tap*COUT], duplicated across both partition halves
    wt = np.ascontiguousarray(
        w.transpose(1, 2, 3, 0).reshape(CIN, 9 * COUT).astype(np.float16))
    wt2 = np.ascontiguousarray(np.concatenate([wt, wt], axis=0))

    ones = np.ones((128, 512), dtype=np.float16)
    # ACT bias: conv bias - 0.5 (SUBTRACT_VALUE) + 3 (hardswish shift)
    bvec = (b.astype(np.float64) + 2.5).astype(np.float32).reshape(128, 1)
    in_maps = [
        {"x": x_de[i], "wt": wt2, "ones": ones, "bvec": bvec}
        for i in range(NCORE)
    ]
    return in_maps


def _run(in_maps, **kw):
    nc = _get_nc()
    return run_bass_kernel_spmd(nc, in_maps, list(range(NCORE)), **kw)


def kernel(x, weight, bias):
    res = _run(_prep(x, weight, bias))
    out = np.empty((B, COUT, HO, WO), dtype=np.float32)
    for i in range(NCORE):
        # device result is [COUT, PER, HO, WO] fp16
        out[PER * i : PER * (i + 1)] = res.results[i]["out"].transpose(
            1, 0, 2, 3)
    return out


# revision 14
# speedup vs baseline: 1.1169x; 1.0518x over previous
"""Trainium2 Bass kernel: strided 3x3 conv (stride 2, pad 1) + bias
+ hardswish + mish, data-parallel over batch across 8 NeuronCores.

Shapes (hardcoded):
  x (16,64,256,256) f32; weight (128,64,3,3); bias (128,)
  out (16,128,128,128) f32

Design:
- Host pre-pads, de-interleaves and fp16-casts x into [128,257,257] per
  core (partition = 2 images x 64 cin): row 0 = top zero pad; per row:
  [128 even cols | 129 odd cols (leading left-pad zero)]. Every conv tap
  reads a CONTIGUOUS 128-wide slice; x DMAs move 2-chunk blocks (33 rows,
  ~17KB contiguous per partition) in ONE dma_start for both images.
- Conv = 9 fp16 tap-matmuls (fp32 PSUM accumulate) into ONE [128,2048]
  PSUM tile per 8-row chunk (img-major, 4 banks). The conv bias, the
  -0.5 shift and hardswish's +3 all ride the ACT bias vector (no bias
  tap matmul). Two images packed in PE row groups (partitions 0-63 /
  64-127, tile_position (0,0)/(64,0)); per tap the order g0:(A,B),
  g1:(A,B) keeps both halves streaming concurrently.
- Tail is 2 ACT + 4 DVE single-instruction passes per chunk, all fp16
  SBUF in accelerated DVE modes (no 1x scalar_tensor_tensor), with ONE
  PSUM read so banks free early:
    q   = Relu(y + b + 2.5)      [ACT, the only PSUM read]
    t6  = min(q,6)/6             [DVE ts 4x]
    qm3 = q - 3                  [DVE ts 4x]
    h6  = qm3*t6                 [DVE tt 2x]  == hardswish, exact
    T   = Sigmoid(g*h6 + a)      [ACT]  ~= tanh(softplus(h6)), fitted
    o   = T*h6                   [DVE tt 2x]  == mish(h6)
  (g,a) least-squares fitted; end-to-end fp16 norm-rel ~7e-3 (budget
  2e-2). sigmoid+relu live in one act table set -> single table load.
  T(c-1)/o(c-1) are emitted after the chunk-c DVE chain so neither ACT
  nor DVE ever stalls on the other.
- Output stays fp16: o written into a 4-chunk staging tile laid out
  [cout, (img, row32, col)] -> ONE HWDGE DMA per 4 chunks (8KB DRAM
  descriptors); host upcasts to fp32. Out DRAM is channel-major
  [COUT, PER, HO, WO]; host transposes on gather.
"""
import numpy as np

import concourse.bass as bass
import concourse.mybir as mybir
import concourse.tile as tile
from concourse import bacc
from concourse.bass_utils import run_bass_kernel_spmd

F32 = mybir.dt.float32
F16 = mybir.dt.float16
AFT = mybir.ActivationFunctionType
ALU = mybir.AluOpType

B, CIN, H, W = 16, 64, 256, 256
COUT = 128
HO, WO = 128, 128
NCORE = 8
PER = B // NCORE          # images per core
R = 8                     # output rows per chunk
NCHUNK = HO // R          # 16
WP = W + 1                # de-interleaved row width (128 even + 129 odd)
XROWS = 4 * R + 1         # input rows per 2-chunk x tile (33)

# mish(h) ~= h*sigmoid(SG*h + SA), fitted over h=hardswish(N(-0.5,1))
SG, SA = 1.41781445, 0.43675223

_CACHE: dict = {}

# inner-column offset into the de-interleaved row, per kj
_KJ_OFF = {0: 128, 1: 0, 2: 129}


def _build():
    nc = bacc.Bacc(None, target_bir_lowering=False)
    x_ext = nc.declare_dram_parameter("x", [PER * CIN, H + 1, WP], F16,
                                      isOutput=False)
    wt_ext = nc.declare_dram_parameter("wt", [128, 9 * COUT], F16,
                                       isOutput=False)
    ones_ext = nc.declare_dram_parameter("ones", [128, 512], F16,
                                         isOutput=False)
    bvec_ext = nc.declare_dram_parameter("bvec", [128, 1], F32,
                                         isOutput=False)
    out_ext = nc.declare_dram_parameter("out", [COUT, PER, HO, WO], F16,
                                        isOutput=True)

    N1 = R * WO            # 1024: one image-chunk
    N2 = PER * N1          # 2048: both images of a chunk

    with tile.TileContext(nc) as tc:
        with (
            tc.tile_pool(name="const", bufs=1) as cpool,
            tc.tile_pool(name="xin", bufs=3) as xpool,
            tc.tile_pool(name="work", bufs=2) as wpool,
            tc.tile_pool(name="h6p", bufs=3) as hpool,
            tc.tile_pool(name="tep", bufs=2) as tpool,
            tc.tile_pool(name="psum", bufs=2, space="PSUM") as ppool,
        ):
            wt_sb = cpool.tile([128, 9 * COUT], F16)
            nc.sync.dma_start(out=wt_sb[:], in_=wt_ext[:])
            ones_sb = cpool.tile([128, 512], F16)
            nc.sync.dma_start(out=ones_sb[:], in_=ones_ext[:])
            bvec_sb = cpool.tile([128, 1], F32)
            nc.sync.dma_start(out=bvec_sb[:], in_=bvec_ext[:])
            sa_sb = cpool.tile([128, 1], F32)
            nc.vector.memset(sa_sb[:], SA)

            # HAM warmup: ~7us of dummy matmuls so the PE clock is at
            # 2.4GHz for the real work and stays busy until the first
            # x DMA lands (no >3.4us idle gap -> no re-throttle).
            warm = ppool.tile([128, N2], F32, tag="pt", name="warm")
            for m in range(32):
                p0 = 64 * (m % 2)
                nc.tensor.matmul(
                    warm[:, (m % 2) * 512 : (m % 2) * 512 + 512],
                    wt_sb[p0 : p0 + 64, 0:COUT],
                    ones_sb[p0 : p0 + 64, :],
                    start=True, stop=True, tile_position=(p0, 0),
                )
            # consume the scratch (also triggers the one act-table load
            # for {sigmoid, relu} before the hot loop)
            wsink = cpool.tile([128, 8], F32)
            nc.scalar.activation(wsink[:], warm[:, 0:8], AFT.Sigmoid)

            te_box = [None]
            h2_box = [None]
            pending = []       # [pair] awaiting sigmoid+mult

            def _tail(g2, h2):
                # One sigmoid + one mult over a 2-chunk pair (N=4096):
                # fewer ACT instructions between the q's means the
                # psum-release chain (PE waits "ACT count >= pos(q)")
                # almost never waits on a T. Lagged one pair so h2 is
                # long since complete when T2 is scheduled.
                T2 = wpool.tile([128, 2 * N2], F16, name=f"T{g2}", tag="T")
                nc.scalar.activation(T2[:], h2[:], AFT.Sigmoid,
                                     scale=SG, bias=sa_sb[:, 0:1])
                if g2 % 2 == 0:
                    te_box[0] = tpool.tile([128, 4 * N2], F16, name="te")
                te = te_box[0]
                tev = te[:].rearrange("p (i c n) -> p i c n", i=PER, c=2)
                nc.vector.tensor_tensor(
                    tev[:, :, g2 % 2, :],
                    T2[:].rearrange("p (i n) -> p i n", i=PER),
                    h2[:].rearrange("p (i n) -> p i n", i=PER),
                    ALU.mult,
                )
                if g2 % 2 == 1:
                    # SWDGE: keeps the dma trigger's sem wait OFF the
                    # ACT queue (a scalar-engine trigger blocks q/T
                    # behind its wait); the gpsimd queue is idle.
                    g4 = g2 // 2
                    nc.gpsimd.dma_start(
                        out=out_ext[:, :, 32 * g4 : 32 * g4 + 32, :],
                        in_=te[:].rearrange("p (i r c) -> p i r c",
                                            i=PER, c=WO),
                    )

            for c in range(NCHUNK):
                if c % 2 == 0:
                    t2 = c // 2
                    xt = xpool.tile([128, XROWS * WP], F16)
                    xt3 = xt[:].rearrange("p (r c) -> p r c", c=WP)
                    nc.sync.dma_start(
                        out=xt3[:, :, :],
                        in_=x_ext[:, 32 * t2 : 32 * t2 + XROWS, :],
                    )
                rbase = 16 * (c % 2)

                pts = ppool.tile([128, N2], F32, tag="pt", name=f"pt{c}")
                for t in range(9):
                    for g in range(2):
                        for i in range(PER):
                            p0 = 64 * i
                            ki, kj = divmod(t, 3)
                            s = rbase + 8 * g + ki
                            off = _KJ_OFF[kj]
                            nc.tensor.matmul(
                                pts[:, i * N1 + g * 512 :
                                       i * N1 + (g + 1) * 512],
                                wt_sb[p0 : p0 + 64,
                                      t * COUT : (t + 1) * COUT],
                                xt3[p0 : p0 + 64, s : s + 7 : 2,
                                    off : off + WO],
                                start=(t == 0), stop=(t == 8),
                                tile_position=(p0, 0),
                            )

                # q = relu(y + b + 2.5); the ONLY psum read -> banks free
                q = wpool.tile([128, N2], F16, name="q", tag="q")
                nc.scalar.activation(q[:], pts[:], AFT.Relu,
                                     bias=bvec_sb[:, 0:1])
                # hardswish from q alone: h6 = (q-3)*min(q,6)/6
                t6 = wpool.tile([128, N2], F16, name="t6", tag="t6")
                nc.vector.tensor_scalar(t6[:], q[:], 6.0, 1.0 / 6.0,
                                        ALU.min, ALU.mult)
                qm3 = wpool.tile([128, N2], F16, name="qm3", tag="qm3")
                nc.vector.tensor_scalar(qm3[:], q[:], -3.0, None, ALU.add)
                # hardswish lands in the 2-chunk pair staging tile
                if c % 2 == 0:
                    h2_box[0] = hpool.tile([128, 2 * N2], F16, name="h2")
                h2 = h2_box[0]
                h2v = h2[:].rearrange("p (i s n) -> p i s n", i=PER, s=2)
                nc.vector.tensor_tensor(
                    h2v[:, :, c % 2, :],
                    qm3[:].rearrange("p (i n) -> p i n", i=PER),
                    t6[:].rearrange("p (i n) -> p i n", i=PER),
                    ALU.mult,
                )
                if c % 2 == 1:
                    pending.append((c // 2, h2))
                    # lag TWO pairs: the scheduler re-sorts the ACT queue
                    # by ready-time, so T2 must be ready before any q it
                    # may get hoisted ahead of
                    if len(pending) > 2:
                        _tail(*pending.pop(0))
            for it in pending:
                _tail(*it)
    nc.compile()
    return nc


def _get_nc():
    if "nc" not in _CACHE:
        _CACHE["nc"] = _build()
    return _CACHE["nc"]


def _prep(x, weight, bias):
    x = np.asarray(x, dtype=np.float32)
    w = np.asarray(weight, dtype=np.float32)
    b = np.asarray(bias, dtype=np.float32)

    # de-interleave + pad + fp16: row 0 = top pad; cols [0:128]=even orig
    # cols, [128]=left pad, [129:257]=odd orig cols 1,3,...,255
    x_de = np.zeros((B, CIN, H + 1, WP), dtype=np.float16)
    x_de[:, :, 1:, 0:128] = x[:, :, :, 0::2]
    x_de[:, :, 1:, 129:257] = x[:, :, :, 1::2]
    x_de = x_de.reshape(NCORE, PER * CIN, H + 1, WP)

    # wt: [cin, tap*COUT], duplicated across both partition halves
    wt = np.ascontiguousarray(
        w.transpose(1, 2, 3, 0).reshape(CIN, 9 * COUT).astype(np.float16))
    wt2 = np.ascontiguousarray(np.concatenate([wt, wt], axis=0))

    ones = np.ones((128, 512), dtype=np.float16)
    # ACT bias: conv bias - 0.5 (SUBTRACT_VALUE) + 3 (hardswish shift)
    bvec = (b.astype(np.float64) + 2.5).astype(np.float32).reshape(128, 1)
    in_maps = [
        {"x": x_de[i], "wt": wt2, "ones": ones, "bvec": bvec}
        for i in range(NCORE)
    ]
    return in_maps


def _run(in_maps, **kw):
    nc = _get_nc()
    return run_bass_kernel_spmd(nc, in_maps, list(range(NCORE)), **kw)


def kernel(x, weight, bias):
    res = _run(_prep(x, weight, bias))
    out = np.empty((B, COUT, HO, WO), dtype=np.float32)
    for i in range(NCORE):
        # device result is [COUT, PER, HO, WO] fp16
        out[PER * i : PER * (i + 1)] = res.results[i]["out"].transpose(
            1, 0, 2, 3)
    return out
